# revision 33
# baseline (speedup 1.0000x reference)
"""Trainium2 Bass kernel for nn_Decoder (causal CNN-GLU decoder with attention).

Computation (per batch):
  x  = shift_right(mel @ W_lin.T + b_lin)
  h1 = causal_cnn_glu(x, w0, b0)              # k=5, D->2D, GLU, residual, /sqrt2
  q  = h1 @ W_attn.T + b_attn
  A  = softmax(q @ enc.T) ; c = A @ (enc + femb)
  h2 = causal_cnn_glu(h1 + c, w1, b1)
  out = h2 @ W_proj.T + b_proj

Sharding: data-parallel over batch B=32 across 8 cores (4 batches/core),
weights replicated.  All activations on-chip are kept feature-major
([D partitions, T free]) so the causal conv taps are just shifted slices
along the free dim and matmul contractions stay on the partition dim.

Scale folding: the two /sqrt(2) are folded into the weights so the GLU
epilogue is exactly two DVE ops per half:
  x' = x/sqrt2  (W_lin,b_lin scaled), conv g-halves scaled by sqrt2,
  conv a-biases scaled by 1/sqrt2, attention context scaled by
  1/(sqrt2*denom) during normalization.
"""

import sys

try:  # prefer the environment's concourse (axon site); fall back to /opt
    import concourse  # noqa: F401
except ImportError:
    sys.path.insert(0, "/opt/trn_rl_repo")

from contextlib import ExitStack  # noqa: E402

import numpy as np  # noqa: E402

import concourse.bass as bass  # noqa: E402
import concourse.mybir as mybir  # noqa: E402
import concourse.tile as tile  # noqa: E402
from concourse import bacc  # noqa: E402
from concourse.masks import make_identity  # noqa: E402

F32 = mybir.dt.float32
F32R = mybir.dt.float32r
BF16 = mybir.dt.bfloat16
AF = mybir.ActivationFunctionType
OP = mybir.AluOpType

B, T_ENC, T_DEC, D, IN = 32, 1024, 2048, 256, 80
NCORES = 8
BPC = B // NCORES
SQRT2 = float(np.sqrt(2.0))
ISQ2 = float(1.0 / np.sqrt(2.0))
SHIFT = 50.0  # softmax stabilization: probs = exp(score - SHIFT)


def _r(ap):
    return ap.bitcast(F32R)


def build_nc(bpc=BPC, t_enc=T_ENC, t_dec=T_DEC, ch=512, num_devices=NCORES,
             loop_n=1, only_phase=None, pb_mode="pool", no_denom=False,
             rotate=False):
    nte = t_enc // 128   # encoder token tiles
    ntd = t_dec // 128   # decoder token tiles
    nch = t_dec // ch    # chunks per batch
    cpt = ch // 128      # 128-token tiles per chunk

    nc = bacc.Bacc("TRN2", target_bir_lowering=False, debug=False,
                   num_devices=num_devices)

    # Host-side prep: mel/out are feature-major (host transposes), and the
    # whole attention front-end is folded on the host:
    #   encsum = (enc + femb) / sqrt2          [token-major]
    #   enctw  = W_attn @ enc^T  (= K'^T)      [d-major]
    #   bshift = enc @ b_attn - SHIFT          [per enc token exp bias]
    # so scores = h1 . K' directly and no q/W_attn work runs on-device.
    encsum_d = nc.dram_tensor("encsum", [bpc, t_enc, D], F32R,
                              kind="ExternalInput")
    enctw_d = nc.dram_tensor("enctw", [bpc, D, t_enc], F32R,
                             kind="ExternalInput")
    bshift_d = nc.dram_tensor("bshift", [bpc, t_enc], F32,
                              kind="ExternalInput")
    mel_d = nc.dram_tensor("mel", [bpc, IN, t_dec], F32R, kind="ExternalInput")
    wlin_d = nc.dram_tensor("wlin", [IN, D], F32R, kind="ExternalInput")
    w0_d = nc.dram_tensor("w0", [128, 5 * 2 * 2 * D], F32R, kind="ExternalInput")
    w1_d = nc.dram_tensor("w1", [128, 5 * 2 * 2 * D], F32R, kind="ExternalInput")
    wproj_d = nc.dram_tensor("wproj", [128, 2 * IN], F32R, kind="ExternalInput")
    bias_d = nc.dram_tensor("bias", [128, 13], F32, kind="ExternalInput")
    out_d = nc.dram_tensor("out", [bpc, IN, t_dec], F32, kind="ExternalOutput")

    with tile.TileContext(nc) as tc, ExitStack() as ctx:
        cpool = ctx.enter_context(tc.tile_pool(name="const", bufs=1))
        stage = ctx.enter_context(tc.tile_pool(name="stage", bufs=4))
        pb = ctx.enter_context(tc.tile_pool(name="perbatch", bufs=1))
        sc = ctx.enter_context(tc.tile_pool(name="scratch", bufs=1))
        sc2 = ctx.enter_context(tc.tile_pool(name="scratch2", bufs=2))
        pmm = ctx.enter_context(
            tc.tile_pool(name="pmm", bufs=4, space=bass.MemorySpace.PSUM))
        pctx = ctx.enter_context(
            tc.tile_pool(name="pctx", bufs=2, space=bass.MemorySpace.PSUM))

        # ---- constants ----
        # (DVE memset can't write f32r; stage in f32 and copy with rounding.)
        ones_f32 = cpool.tile([128, 1], F32, tag="ones_f32")
        nc.vector.memset(ones_f32[:], 1.0)
        ones_col = cpool.tile([128, 1], F32R, tag="ones")
        nc.vector.tensor_copy(ones_col[:], ones_f32[:])
        zero4 = cpool.tile([128, 4], F32, tag="zero4")
        nc.vector.memset(zero4[:], 0.0)

        wlin = cpool.tile([IN, D], F32R, tag="wlin")
        w0 = cpool.tile([128, 5 * 2 * 2 * D], F32R, tag="w0")
        w1 = cpool.tile([128, 5 * 2 * 2 * D], F32R, tag="w1")
        wproj = cpool.tile([128, 2 * IN], F32R, tag="wproj")
        bias = cpool.tile([128, 13], F32, tag="bias")

        def load_weights_early():
            nc.sync.dma_start(out=wlin[:], in_=wlin_d[:])
            nc.sync.dma_start(out=bias[:], in_=bias_d[:])

        def load_weights_mid():
            nc.sync.dma_start(out=w0[:], in_=w0_d[:])

        def load_weights_late():
            nc.sync.dma_start(out=wproj[:], in_=wproj_d[:])
            nc.sync.dma_start(out=w1[:], in_=w1_d[:])

        def bcol(j):
            return bias[:, j:j + 1]

        def conv_glu(w_sb, ba0, bg0, in_buf, base, out_ap_fn, resid_ap_fn):
            """One causal-conv+GLU chunk.  in_buf: [128, 2, T+4] padded buffer.
            out_ap_fn(i) / resid_ap_fn(i) give [128, ch] APs for d-tile i."""
            s_tiles = {}
            for j in (2, 3, 0, 1):
                pc = pmm.tile([128, ch], F32, tag="mm")
                k = 0
                for t in range(5):
                    for i in range(2):
                        col = (t * 2 + i) * (2 * D) + j * 128
                        nc.tensor.matmul(
                            pc[:],
                            _r(w_sb[:, col:col + 128]),
                            _r(in_buf[:, i, base + t:base + t + ch]),
                            start=(k == 0), stop=(k == 9))
                        k += 1
                if j >= 2:
                    s = sc2.tile([128, ch], F32, tag=f"sig{j - 2}", name=f"sig{j - 2}")
                    nc.scalar.activation(s[:], pc[:], AF.Sigmoid,
                                         bias=bcol(bg0 + (j - 2)))
                    s_tiles[j - 2] = s
                else:
                    o = out_ap_fn(j)
                    nc.vector.scalar_tensor_tensor(
                        o, pc[:], bcol(ba0 + j), s_tiles[j][:],
                        op0=OP.add, op1=OP.mult)
                    nc.vector.tensor_add(o, o, resid_ap_fn(j))

        def prep_mel(b, melT, x_buf):
            """mel arrives feature-major from the host: chunked direct DMA."""
            with nc.named_scope(f"prepmel{b}"):
                if b == 0:
                    load_weights_early()
                nc.vector.tensor_copy(melT[:, 0:1], zero4[0:IN, 0:1])
                for i in range(2):
                    nc.vector.tensor_copy(x_buf[:, i, 0:4], zero4[:])
                # per-chunk DMAs so the first linear matmul starts early
                for c in range(nch):
                    base = c * ch
                    nc.sync.dma_start(
                        out=melT[:, 1 + base:1 + base + ch],
                        in_=mel_d[b][:, base:base + ch])
                if b == 0:
                    load_weights_mid()

        def prep_enc(b, encT, encsum, bsh):
            # All three come straight from host-prepped DRAM.  Token order is
            # natural everywhere: probs group j partition r <-> token j*128+r
            # in both the scores stationary (encT free dim) and the context
            # stationary (encsum partition dim).
            with nc.named_scope(f"prepenc{b}"):
                nc.sync.dma_start(
                    out=encT[:],
                    in_=enctw_d[b].rearrange("(i p) t -> p i t", p=128))
                nc.sync.dma_start(
                    out=encsum[:],
                    in_=encsum_d[b].rearrange("(n p) d -> p n d", p=128))
                nc.sync.dma_start(
                    out=bsh[:], in_=bshift_d[b].rearrange("(n p) -> p n", p=128))
                if b == 0:
                    load_weights_late()

        def body_emit():
            melTs, x_bufs, h1_bufs = {}, {}, {}

            def alloc_mel(b):
                melTs[b] = pb.tile([IN, t_dec + 1], F32R, tag="melT", name="melT")
                x_bufs[b] = pb.tile([128, 2, t_dec + 4], F32R, tag="x_buf",
                                    name="x_buf")

            def emit_phA(b):
                """linear + conv0 for all chunks of batch b (sigmoid table)."""
                melT = melTs.pop(b)
                x_buf = x_bufs[b]
                h1_bufs[b] = pb.tile([128, 2, t_dec], F32R, tag="h1_buf",
                                     name="h1_buf")
                h1_buf = h1_bufs[b]
                with nc.named_scope(f"phA_{b}"):
                    def lin(c):
                        base = c * ch
                        for i in range(2):
                            px = pmm.tile([128, ch], F32, tag="mm", name="px")
                            nc.tensor.matmul(px[:],
                                             _r(wlin[:, i * 128:(i + 1) * 128]),
                                             _r(melT[:, base:base + ch]),
                                             start=True, stop=True)
                            # evacuate on DVE: ACT is busy with sigmoid/exp
                            # tables and stalls the conv/scores that follow
                            nc.vector.tensor_scalar_add(
                                x_buf[:, i, 4 + base:4 + base + ch],
                                px[:], bcol(0 + i))
                        if c == 0:
                            # x[0] must be exactly 0 (shift pad), not b_lin
                            for i2 in range(2):
                                nc.vector.tensor_copy(x_buf[:, i2, 4:5],
                                                      zero4[:, 0:1])
                    # interleave linear with conv0 so PE isn't waiting on the
                    # DVE evacuation of the very first x chunks at startup
                    lin(0)
                    lin(1)
                    for c in range(nch):
                        base = c * ch
                        conv_glu(w0, 2, 4, x_buf, base,
                                 lambda i: h1_buf[:, i, base:base + ch],
                                 lambda i: x_buf[:, i, 4 + base:4 + base + ch])
                        if c + 2 < nch:
                            lin(c + 2)

            if only_phase in ("B", "C"):
                # isolated-phase builds still need weights + dummy producers
                load_weights_early()
                load_weights_mid()
                load_weights_late()
            alloc_mel(0)
            if only_phase in (None, "A"):
                prep_mel(0, melTs[0], x_bufs[0])
                emit_phA(0)

            for b in range(bpc):
                if not rotate and b > 0 and only_phase in (None, "A"):
                    emit_phA(b)
                encT = pb.tile([128, 2, t_enc], F32R, tag="encT", name="encT")
                encsum = pb.tile([128, nte, D], F32R, tag="encsum", name="encsum")
                bsh = pb.tile([128, nte], F32, tag="bsh", name="bsh")
                hA_buf = pb.tile([128, 2, t_dec + 4], F32R, tag="hA_buf",
                                 name="hA_buf")
                if only_phase == "B":
                    h1_bufs[b] = pb.tile([128, 2, t_dec], F32R, tag="h1_buf",
                                         name="h1_buf")
                h1_buf = h1_bufs.get(b)

                if only_phase in (None, "B"):
                    prep_enc(b, encT, encsum, bsh)
                # next batch's mel prep hides under phB/phC of this batch
                if b + 1 < bpc:
                    alloc_mel(b + 1)
                    if only_phase in (None, "A"):
                        prep_mel(b + 1, melTs[b + 1], x_bufs[b + 1])

                # hA zero pads (hA_buf slot frees once conv1 of b-1 is done)
                for i in range(2):
                    nc.vector.tensor_copy(hA_buf[:, i, 0:4], zero4[:])
                if only_phase == "B":
                    for i in range(2):
                        nc.vector.tensor_copy(h1_buf[:, i, 0:4], zero4[:])
                if only_phase == "C":
                    for i in range(2):
                        nc.vector.tensor_copy(hA_buf[:, i, 4:8], zero4[:])

                # ---- phase B: attention for all chunks (exp table) ----
                # scores = h1 . K' with K' host-folded (enc @ W_attn^T), so
                # the moving operand is h1 directly and there is no q step.
                def scores_exp(c):
                    base = c * ch
                    probs = sc.tile([128, nte, ch], F32R, tag="probs", name="probs")
                    for j in range(nte):
                        ps = pmm.tile([128, ch], F32, tag="mm", name="ps")
                        for i in range(2):
                            nc.tensor.matmul(ps[:],
                                             _r(encT[:, i, j * 128:(j + 1) * 128]),
                                             _r(h1_buf[:, i, base:base + ch]),
                                             start=(i == 0), stop=(i == 1))
                        nc.scalar.activation(probs[:, j, :], ps[:], AF.Exp,
                                             bias=bsh[:, j:j + 1])
                    # probs sum over the 8 token tiles runs on Pool + DVE
                    # (split so neither exceeds its per-chunk budget); PE does
                    # a single 512-col ones-matmul for the partition reduction
                    # instead of eight — and that matmul is deferred into the
                    # next chunk's scores so it never waits on this chain.
                    psum_t = sc.tile([128, ch], F32R, tag=f"psumt{c % 2}",
                                     name="psum_t")
                    if not no_denom:
                        def pf(j):
                            return probs[:, j, :]
                        dsum_t = sc.tile([128, ch], F32R, tag=f"dsumt{c % 2}",
                                         name="dsum_t")
                        nc.gpsimd.tensor_add(psum_t[:], pf(0), pf(1))
                        nc.gpsimd.tensor_add(psum_t[:], psum_t[:], pf(2))
                        nc.gpsimd.tensor_add(psum_t[:], psum_t[:], pf(3))
                        nc.vector.tensor_add(dsum_t[:], pf(4), pf(5))
                        nc.vector.tensor_add(dsum_t[:], dsum_t[:], pf(6))
                        nc.vector.tensor_add(dsum_t[:], dsum_t[:], pf(7))
                        nc.gpsimd.tensor_add(psum_t[:], psum_t[:], dsum_t[:])
                    return probs, psum_t

                def attn_ctx(c, probs):
                    pc0 = pctx.tile([128, ch], F32, tag="c0", name="pc0")
                    pc1 = pctx.tile([128, ch], F32, tag="c1", name="pc1")
                    for j in range(nte):
                        pr = probs[:, j, :]
                        nc.tensor.matmul(pc0[:], encsum[:, j, 0:128], pr,
                                         start=(j == 0), stop=(j == nte - 1))
                        nc.tensor.matmul(pc1[:], encsum[:, j, 128:256], pr,
                                         start=(j == 0), stop=(j == nte - 1))
                    return [pc0, pc1]

                def finish_pd(c, psum_t):
                    den_r = sc.tile([1, ch], F32, tag=f"den{c % 2}", name="den")
                    if not no_denom:
                        pd = pmm.tile([1, ch], F32, tag="mm", name="pd")
                        nc.tensor.matmul(pd[:], ones_col[:], psum_t[:],
                                         start=True, stop=True)
                        nc.vector.reciprocal(den_r[:], pd[:])
                    return den_r

                def finish_epi(c, den_r, pcx):
                    base = c * ch
                    rep = sc.tile([128, ch], F32, tag=f"rep{c % 2}", name="rep")
                    if no_denom:
                        nc.vector.memset(rep[:], 1.0)
                    else:
                        nc.gpsimd.partition_broadcast(rep[:], den_r[:])
                    for i in range(2):
                        tmp = sc.tile([128, ch], F32, tag=f"tmp{i}", name=f"tmp{i}")
                        nc.vector.tensor_tensor(tmp[:], pcx[i][:], rep[:], op=OP.mult)
                        # hA' = h1/sqrt2 + ctx_unnorm * (isq2/denom)
                        nc.vector.scalar_tensor_tensor(
                            hA_buf[:, i, 4 + base:4 + base + ch],
                            h1_buf[:, i, base:base + ch], ISQ2, tmp[:],
                            op0=OP.mult, op1=OP.add)

                pend = None
                if only_phase in (None, "B"):
                    with nc.named_scope(f"phB_{b}"):
                        # The denominator finish of chunk c-1 rides between
                        # scores(c) and ctx(c) so PE never waits on the sum; the
                        # last chunk's finish is deferred behind phC's first conv.
                        for c in range(nch):
                            probs, psum_t = scores_exp(c)
                            if pend is not None:
                                den_prev = finish_pd(pend[0], pend[1])
                            pcx = attn_ctx(c, probs)
                            if pend is not None:
                                finish_epi(pend[0], den_prev, pend[2])
                            pend = (c, psum_t, pcx)

                # rotated schedule: conv0 of batch b+1 is emitted here, between
                # phB(b) and phC(b).  phC(b) depends on phB(b)'s DVE epilogue
                # (hA), so conv0(b+1) gives PE independent work to chew on while
                # that drains; ACT table order stays exp -> sigmoid -> sigmoid.
                if rotate and b + 1 < bpc and only_phase in (None, "A"):
                    emit_phA(b + 1)

                # ---- phase C: conv1 + proj for all chunks (sigmoid table) ----
                # proj(c-1) is emitted after conv1(c) so PE never waits on the
                # GLU DVE epilogue of chunk c before starting useful work.
                def proj_and_out(c, h2, last=False):
                    base = c * ch
                    pp = pmm.tile([IN, ch], F32, tag="mm", name="pp")
                    for kk in range(2):
                        nc.tensor.matmul(pp[:], _r(wproj[:, kk * IN:(kk + 1) * IN]),
                                         _r(h2[kk][:]), start=(kk == 0),
                                         stop=(kk == 1))
                    proj = sc2.tile([IN, ch], F32, tag="proj", name="proj")
                    nc.scalar.activation(proj[:], pp[:], AF.Identity,
                                         bias=bias[0:IN, 12:13])
                    # out stays feature-major; the host transposes it back
                    nc.sync.dma_start(out=out_d[b][:, base:base + ch],
                                      in_=proj[:])

                if only_phase in (None, "C"):
                    with nc.named_scope(f"phC_{b}"):
                        h2_prev = None
                        for c in range(nch):
                            base = c * ch
                            h2 = [sc2.tile([128, ch], F32R, tag=f"h2_{i}",
                                           name=f"h2_{i}") for i in range(2)]
                            conv_glu(w1, 6, 8, hA_buf, base,
                                     lambda i: h2[i][:],
                                     lambda i: hA_buf[:, i, 4 + base:4 + base + ch])
                            if c == 0 and pend is not None:
                                den_last = finish_pd(pend[0], pend[1])
                                finish_epi(pend[0], den_last, pend[2])
                            if h2_prev is not None:
                                proj_and_out(c - 1, h2_prev)
                            h2_prev = h2
                        proj_and_out(nch - 1, h2_prev, last=(b == bpc - 1))
        import contextlib
        loop_cm = (tc.For_i(0, loop_n, 1, hint_engines=(mybir.EngineType.PE,))
                   if loop_n > 1 else contextlib.nullcontext())
        with loop_cm:
            body_emit()

    nc.compile()
    return nc


def prep_weights(W_lin, b_lin, conv_w0, conv_b0, conv_w1, conv_b1,
                 W_attn, b_attn, W_proj, b_proj):
    def prep_conv(w):
        ws = w.astype(np.float32).copy()
        ws[D:] *= SQRT2                       # g-half
        # [512, 256, 5] -> [p, t, i, o] -> [128, 5*2*512]
        arr = ws.transpose(1, 2, 0).reshape(2, 128, 5, 2 * D).transpose(1, 2, 0, 3)
        return np.ascontiguousarray(arr.reshape(128, 5 * 2 * 2 * D))

    wlin_h = np.ascontiguousarray(W_lin.T * ISQ2).astype(np.float32)
    wproj_h = np.ascontiguousarray(
        W_proj.T.reshape(2, 128, IN).transpose(1, 0, 2).reshape(128, 2 * IN)
    ).astype(np.float32)

    bias_h = np.zeros((128, 13), np.float32)
    bias_h[:, 0] = b_lin[0:128] * ISQ2
    bias_h[:, 1] = b_lin[128:256] * ISQ2
    bias_h[:, 2] = conv_b0[0:128] * ISQ2      # a-half biases scaled
    bias_h[:, 3] = conv_b0[128:256] * ISQ2
    bias_h[:, 4] = conv_b0[256:384]           # g-half biases unscaled
    bias_h[:, 5] = conv_b0[384:512]
    bias_h[:, 6] = conv_b1[0:128] * ISQ2
    bias_h[:, 7] = conv_b1[128:256] * ISQ2
    bias_h[:, 8] = conv_b1[256:384]
    bias_h[:, 9] = conv_b1[384:512]
    bias_h[0:IN, 12] = b_proj

    return {
        "wlin": wlin_h, "w0": prep_conv(conv_w0), "w1": prep_conv(conv_w1),
        "wproj": wproj_h, "bias": bias_h,
    }


def prep_attn(enc, femb, W_attn, b_attn):
    """Host-folded attention front-end for one shard of batches."""
    enc = np.asarray(enc, np.float32)
    encsum = ((enc + np.asarray(femb, np.float32)) * ISQ2).astype(np.float32)
    # enctw[b][f, s] = sum_d W_attn[d, f] enc[b, s, d]  (= (enc @ W_attn)^T,
    # d-major) so that scores[s, t] = sum_f enctw[f, s] h1[f, t]
    enctw = np.einsum("df,bsd->bfs", np.asarray(W_attn, np.float32), enc,
                      optimize=True).astype(np.float32)
    bshift = (enc @ np.asarray(b_attn, np.float32) - SHIFT).astype(np.float32)
    return {"encsum": np.ascontiguousarray(encsum),
            "enctw": np.ascontiguousarray(enctw),
            "bshift": np.ascontiguousarray(bshift)}


_NC = None


def _get_nc():
    global _NC
    if _NC is None:
        _NC = build_nc()
    return _NC


def kernel(encoder_outputs, first_embedding, mel_inputs,
           W_lin, b_lin, conv_w0, conv_b0, conv_w1, conv_b1,
           W_attn, b_attn, W_proj, b_proj):
    from concourse.bass_utils import run_bass_kernel_spmd

    nc = _get_nc()
    w = prep_weights(W_lin, b_lin, conv_w0, conv_b0, conv_w1, conv_b1,
                     W_attn, b_attn, W_proj, b_proj)
    enc = np.asarray(encoder_outputs, np.float32)
    femb = np.asarray(first_embedding, np.float32)
    mel = np.asarray(mel_inputs, np.float32)
    in_maps = []
    for c in range(NCORES):
        sl = slice(c * BPC, (c + 1) * BPC)
        in_maps.append({**prep_attn(enc[sl], femb[sl], W_attn, b_attn),
                        "mel": np.ascontiguousarray(mel[sl].transpose(0, 2, 1)),
                        **w})
    res = run_bass_kernel_spmd(nc, in_maps, list(range(NCORES)))
    out = np.concatenate([res.results[i]["out"] for i in range(NCORES)], axis=0)
    return np.ascontiguousarray(out.transpose(0, 2, 1))



# revision 37
# speedup vs baseline: 1.1338x; 1.1338x over previous
"""Trainium2 Bass kernel for nn_Decoder (causal CNN-GLU decoder with attention).

Computation (per batch):
  x  = shift_right(mel @ W_lin.T + b_lin)
  h1 = causal_cnn_glu(x, w0, b0)              # k=5, D->2D, GLU, residual, /sqrt2
  q  = h1 @ W_attn.T + b_attn
  A  = softmax(q @ enc.T) ; c = A @ (enc + femb)
  h2 = causal_cnn_glu(h1 + c, w1, b1)
  out = h2 @ W_proj.T + b_proj

Sharding: data-parallel over batch B=32 across 8 cores (4 batches/core),
weights replicated.  All activations on-chip are kept feature-major
([D partitions, T free]) so the causal conv taps are just shifted slices
along the free dim and matmul contractions stay on the partition dim.

Scale folding: the two /sqrt(2) are folded into the weights so the GLU
epilogue is exactly two DVE ops per half:
  x' = x/sqrt2  (W_lin,b_lin scaled), conv g-halves scaled by sqrt2,
  conv a-biases scaled by 1/sqrt2, attention context scaled by
  1/(sqrt2*denom) during normalization.
"""

import sys

try:  # prefer the environment's concourse (axon site); fall back to /opt
    import concourse  # noqa: F401
except ImportError:
    sys.path.insert(0, "/opt/trn_rl_repo")

from contextlib import ExitStack  # noqa: E402

import numpy as np  # noqa: E402

import concourse.bass as bass  # noqa: E402
import concourse.mybir as mybir  # noqa: E402
import concourse.tile as tile  # noqa: E402
from concourse import bacc  # noqa: E402
from concourse.masks import make_identity  # noqa: E402

F32 = mybir.dt.float32
F32R = mybir.dt.float32r
BF16 = mybir.dt.bfloat16
AF = mybir.ActivationFunctionType
OP = mybir.AluOpType

B, T_ENC, T_DEC, D, IN = 32, 1024, 2048, 256, 80
NCORES = 8
BPC = B // NCORES
SQRT2 = float(np.sqrt(2.0))
ISQ2 = float(1.0 / np.sqrt(2.0))
SHIFT = 50.0  # softmax stabilization: probs = exp(score - SHIFT)


def _r(ap):
    return ap.bitcast(F32R)


def build_nc(bpc=BPC, t_enc=T_ENC, t_dec=T_DEC, ch=512, num_devices=NCORES,
             loop_n=1, only_phase=None, pb_mode="pool", no_denom=False,
             rotate=False, denom_pe=False):
    nte = t_enc // 128   # encoder token tiles
    ntd = t_dec // 128   # decoder token tiles
    nch = t_dec // ch    # chunks per batch
    cpt = ch // 128      # 128-token tiles per chunk

    nc = bacc.Bacc("TRN2", target_bir_lowering=False, debug=False,
                   num_devices=num_devices)

    # Host-side prep: mel/out are feature-major (host transposes), and the
    # whole attention front-end is folded on the host:
    #   encsum = (enc + femb) / sqrt2          [token-major]
    #   enctw  = W_attn @ enc^T  (= K'^T)      [d-major]
    #   bshift = enc @ b_attn - SHIFT          [per enc token exp bias]
    # so scores = h1 . K' directly and no q/W_attn work runs on-device.
    # All enc-side tensors are host-permuted to the SAME p-outer token order
    # (token = p*nte + n) so every DMA lands as 128 partitions x contiguous
    # 8KB runs (descriptor-light); attention is permutation-invariant as long
    # as scores/context/bshift agree on the order.
    encsum_d = nc.dram_tensor("encsum", [bpc, t_enc, D], F32R,
                              kind="ExternalInput")
    enctw_d = nc.dram_tensor("enctw", [bpc, 128, 2 * t_enc], F32R,
                             kind="ExternalInput")
    bshift_d = nc.dram_tensor("bshift", [bpc, 128, t_enc // 128], F32,
                              kind="ExternalInput")
    mel_d = nc.dram_tensor("mel", [bpc, IN, t_dec], F32R, kind="ExternalInput")
    wlin_d = nc.dram_tensor("wlin", [IN, D], F32R, kind="ExternalInput")
    w0_d = nc.dram_tensor("w0", [128, 5 * 2 * 2 * D], F32R, kind="ExternalInput")
    w1_d = nc.dram_tensor("w1", [128, 5 * 2 * 2 * D], F32R, kind="ExternalInput")
    wproj_d = nc.dram_tensor("wproj", [128, 2 * IN], F32R, kind="ExternalInput")
    bias_d = nc.dram_tensor("bias", [128, 13], F32, kind="ExternalInput")
    out_d = nc.dram_tensor("out", [bpc, IN, t_dec], F32, kind="ExternalOutput")

    with tile.TileContext(nc) as tc, ExitStack() as ctx:
        cpool = ctx.enter_context(tc.tile_pool(name="const", bufs=1))
        stage = ctx.enter_context(tc.tile_pool(name="stage", bufs=4))
        pb = ctx.enter_context(tc.tile_pool(name="perbatch", bufs=1))
        sc = ctx.enter_context(tc.tile_pool(name="scratch", bufs=1))
        sc2 = ctx.enter_context(tc.tile_pool(name="scratch2", bufs=2))
        pmm = ctx.enter_context(
            tc.tile_pool(name="pmm", bufs=4, space=bass.MemorySpace.PSUM))
        pctx = ctx.enter_context(
            tc.tile_pool(name="pctx", bufs=2, space=bass.MemorySpace.PSUM))

        # ---- constants ----
        # (DVE memset can't write f32r; stage in f32 and copy with rounding.)
        ones_f32 = cpool.tile([128, 1], F32, tag="ones_f32")
        nc.vector.memset(ones_f32[:], 1.0)
        ones_col = cpool.tile([128, 1], F32R, tag="ones")
        nc.vector.tensor_copy(ones_col[:], ones_f32[:])
        zero4 = cpool.tile([128, 4], F32, tag="zero4")
        nc.vector.memset(zero4[:], 0.0)

        wlin = cpool.tile([IN, D], F32R, tag="wlin")
        w0 = cpool.tile([128, 5 * 2 * 2 * D], F32R, tag="w0")
        w1 = cpool.tile([128, 5 * 2 * 2 * D], F32R, tag="w1")
        wproj = cpool.tile([128, 2 * IN], F32R, tag="wproj")
        bias = cpool.tile([128, 13], F32, tag="bias")

        def load_weights_early():
            nc.sync.dma_start(out=wlin[:], in_=wlin_d[:])
            nc.sync.dma_start(out=bias[:], in_=bias_d[:])

        def load_weights_mid():
            nc.sync.dma_start(out=w0[:], in_=w0_d[:])

        def load_weights_late():
            nc.sync.dma_start(out=wproj[:], in_=wproj_d[:])
            nc.sync.dma_start(out=w1[:], in_=w1_d[:])

        def bcol(j):
            return bias[:, j:j + 1]

        def conv_glu(w_sb, ba0, bg0, in_buf, base, out_ap_fn, resid_ap_fn):
            """One causal-conv+GLU chunk.  in_buf: [128, 2, T+4] padded buffer.
            out_ap_fn(i) / resid_ap_fn(i) give [128, ch] APs for d-tile i."""
            s_tiles = {}
            for j in (2, 3, 0, 1):
                pc = pmm.tile([128, ch], F32, tag="mm")
                k = 0
                for t in range(5):
                    for i in range(2):
                        col = (t * 2 + i) * (2 * D) + j * 128
                        nc.tensor.matmul(
                            pc[:],
                            _r(w_sb[:, col:col + 128]),
                            _r(in_buf[:, i, base + t:base + t + ch]),
                            start=(k == 0), stop=(k == 9))
                        k += 1
                if j >= 2:
                    s = sc2.tile([128, ch], F32, tag=f"sig{j - 2}", name=f"sig{j - 2}")
                    nc.scalar.activation(s[:], pc[:], AF.Sigmoid,
                                         bias=bcol(bg0 + (j - 2)))
                    s_tiles[j - 2] = s
                else:
                    o = out_ap_fn(j)
                    nc.vector.scalar_tensor_tensor(
                        o, pc[:], bcol(ba0 + j), s_tiles[j][:],
                        op0=OP.add, op1=OP.mult)
                    nc.vector.tensor_add(o, o, resid_ap_fn(j))

        def prep_mel(b, melT, x_buf):
            """mel arrives feature-major from the host: chunked direct DMA."""
            with nc.named_scope(f"prepmel{b}"):
                if b == 0:
                    load_weights_early()
                nc.vector.tensor_copy(melT[:, 0:1], zero4[0:IN, 0:1])
                for i in range(2):
                    nc.vector.tensor_copy(x_buf[:, i, 0:4], zero4[:])
                # per-chunk DMAs so the first linear matmul starts early
                for c in range(nch):
                    base = c * ch
                    nc.sync.dma_start(
                        out=melT[:, 1 + base:1 + base + ch],
                        in_=mel_d[b][:, base:base + ch])
                if b == 0:
                    load_weights_mid()

        def prep_enc(b, encT, encsum, bsh):
            # All three come straight from host-prepped DRAM.  Token order is
            # natural everywhere: probs group j partition r <-> token j*128+r
            # in both the scores stationary (encT free dim) and the context
            # stationary (encsum partition dim).
            with nc.named_scope(f"prepenc{b}"):
                nc.sync.dma_start(
                    out=encT[:],
                    in_=enctw_d[b].rearrange("p (i t) -> p i t", i=2))
                nc.sync.dma_start(
                    out=encsum[:],
                    in_=encsum_d[b].rearrange("(p n) d -> p n d", p=128))
                nc.sync.dma_start(out=bsh[:], in_=bshift_d[b])
                if b == 0:
                    load_weights_late()

        def body_emit():
            melTs, x_bufs, h1_bufs = {}, {}, {}

            def alloc_mel(b):
                melTs[b] = pb.tile([IN, t_dec + 1], F32R, tag="melT", name="melT")
                x_bufs[b] = pb.tile([128, 2, t_dec + 4], F32R, tag="x_buf",
                                    name="x_buf")

            def emit_phA(b):
                """linear + conv0 for all chunks of batch b (sigmoid table)."""
                melT = melTs.pop(b)
                x_buf = x_bufs[b]
                h1_bufs[b] = pb.tile([128, 2, t_dec], F32R, tag="h1_buf",
                                     name="h1_buf")
                h1_buf = h1_bufs[b]
                with nc.named_scope(f"phA_{b}"):
                    def lin(c):
                        base = c * ch
                        for i in range(2):
                            px = pmm.tile([128, ch], F32, tag="mm", name="px")
                            nc.tensor.matmul(px[:],
                                             _r(wlin[:, i * 128:(i + 1) * 128]),
                                             _r(melT[:, base:base + ch]),
                                             start=True, stop=True)
                            # evacuate on DVE: ACT is busy with sigmoid/exp
                            # tables and stalls the conv/scores that follow
                            nc.vector.tensor_scalar_add(
                                x_buf[:, i, 4 + base:4 + base + ch],
                                px[:], bcol(0 + i))
                        if c == 0:
                            # x[0] must be exactly 0 (shift pad), not b_lin
                            for i2 in range(2):
                                nc.vector.tensor_copy(x_buf[:, i2, 4:5],
                                                      zero4[:, 0:1])
                    # interleave linear with conv0 so PE isn't waiting on the
                    # DVE evacuation of the very first x chunks at startup
                    lin(0)
                    lin(1)
                    for c in range(nch):
                        base = c * ch
                        conv_glu(w0, 2, 4, x_buf, base,
                                 lambda i: h1_buf[:, i, base:base + ch],
                                 lambda i: x_buf[:, i, 4 + base:4 + base + ch])
                        if c + 2 < nch:
                            lin(c + 2)

            if only_phase in ("B", "C"):
                # isolated-phase builds still need weights + dummy producers
                load_weights_early()
                load_weights_mid()
                load_weights_late()
            alloc_mel(0)
            if only_phase in (None, "A"):
                prep_mel(0, melTs[0], x_bufs[0])
                emit_phA(0)

            for b in range(bpc):
                if not rotate and b > 0 and only_phase in (None, "A"):
                    emit_phA(b)
                encT = pb.tile([128, 2, t_enc], F32R, tag="encT", name="encT")
                encsum = pb.tile([128, nte, D], F32R, tag="encsum", name="encsum")
                bsh = pb.tile([128, nte], F32, tag="bsh", name="bsh")
                hA_buf = pb.tile([128, 2, t_dec + 4], F32R, tag="hA_buf",
                                 name="hA_buf")
                if only_phase == "B":
                    h1_bufs[b] = pb.tile([128, 2, t_dec], F32R, tag="h1_buf",
                                         name="h1_buf")
                h1_buf = h1_bufs.get(b)

                if only_phase in (None, "B"):
                    prep_enc(b, encT, encsum, bsh)
                # next batch's mel prep hides under phB/phC of this batch
                if b + 1 < bpc:
                    alloc_mel(b + 1)
                    if only_phase in (None, "A"):
                        prep_mel(b + 1, melTs[b + 1], x_bufs[b + 1])

                # hA zero pads (hA_buf slot frees once conv1 of b-1 is done)
                for i in range(2):
                    nc.vector.tensor_copy(hA_buf[:, i, 0:4], zero4[:])
                if only_phase == "B":
                    for i in range(2):
                        nc.vector.tensor_copy(h1_buf[:, i, 0:4], zero4[:])
                if only_phase == "C":
                    for i in range(2):
                        nc.vector.tensor_copy(hA_buf[:, i, 4:8], zero4[:])

                # ---- phase B: attention for all chunks (exp table) ----
                # scores = h1 . K' with K' host-folded (enc @ W_attn^T), so
                # the moving operand is h1 directly and there is no q step.
                def scores_exp(c):
                    base = c * ch
                    probs = sc.tile([128, nte, ch], F32R, tag="probs", name="probs")
                    for j in range(nte):
                        ps = pmm.tile([128, ch], F32, tag="mm", name="ps")
                        for i in range(2):
                            nc.tensor.matmul(ps[:],
                                             _r(encT[:, i, j * 128:(j + 1) * 128]),
                                             _r(h1_buf[:, i, base:base + ch]),
                                             start=(i == 0), stop=(i == 1))
                        nc.scalar.activation(probs[:, j, :], ps[:], AF.Exp,
                                             bias=bsh[:, j:j + 1])
                    # probs sum over the 8 token tiles runs on Pool + DVE
                    # (split so neither exceeds its per-chunk budget); PE does
                    # a single 512-col ones-matmul for the partition reduction
                    # instead of eight — and that matmul is deferred into the
                    # next chunk's scores so it never waits on this chain.
                    psum_t = sc.tile([128, ch], F32R, tag=f"psumt{c % 2}",
                                     name="psum_t")
                    if not no_denom:
                        def pf(j):
                            return probs[:, j, :]
                        dsum_t = sc.tile([128, ch], F32R, tag=f"dsumt{c % 2}",
                                         name="dsum_t")
                        nc.gpsimd.tensor_add(psum_t[:], pf(0), pf(1))
                        nc.gpsimd.tensor_add(psum_t[:], psum_t[:], pf(2))
                        nc.gpsimd.tensor_add(psum_t[:], psum_t[:], pf(3))
                        nc.vector.tensor_add(dsum_t[:], pf(4), pf(5))
                        nc.vector.tensor_add(dsum_t[:], dsum_t[:], pf(6))
                        nc.vector.tensor_add(dsum_t[:], dsum_t[:], pf(7))
                        nc.gpsimd.tensor_add(psum_t[:], psum_t[:], dsum_t[:])
                    return probs, psum_t

                def attn_ctx(c, probs):
                    pc0 = pctx.tile([128, ch], F32, tag="c0", name="pc0")
                    pc1 = pctx.tile([128, ch], F32, tag="c1", name="pc1")
                    for j in range(nte):
                        pr = probs[:, j, :]
                        nc.tensor.matmul(pc0[:], encsum[:, j, 0:128], pr,
                                         start=(j == 0), stop=(j == nte - 1))
                        nc.tensor.matmul(pc1[:], encsum[:, j, 128:256], pr,
                                         start=(j == 0), stop=(j == nte - 1))
                    return [pc0, pc1]

                def finish_pd(c, psum_t):
                    den_r = sc.tile([1, ch], F32, tag=f"den{c % 2}", name="den")
                    if not no_denom:
                        pd = pmm.tile([1, ch], F32, tag="mm", name="pd")
                        nc.tensor.matmul(pd[:], ones_col[:], psum_t[:],
                                         start=True, stop=True)
                        nc.vector.reciprocal(den_r[:], pd[:])
                    return den_r

                def finish_epi(c, den_r, pcx):
                    base = c * ch
                    rep = sc.tile([128, ch], F32, tag=f"rep{c % 2}", name="rep")
                    if no_denom:
                        nc.vector.memset(rep[:], 1.0)
                    else:
                        nc.gpsimd.partition_broadcast(rep[:], den_r[:])
                    for i in range(2):
                        tmp = sc.tile([128, ch], F32, tag=f"tmp{i}", name=f"tmp{i}")
                        nc.vector.tensor_tensor(tmp[:], pcx[i][:], rep[:], op=OP.mult)
                        # hA' = h1/sqrt2 + ctx_unnorm * (isq2/denom)
                        nc.vector.scalar_tensor_tensor(
                            hA_buf[:, i, 4 + base:4 + base + ch],
                            h1_buf[:, i, base:base + ch], ISQ2, tmp[:],
                            op0=OP.mult, op1=OP.add)

                pend = None
                if only_phase in (None, "B"):
                    with nc.named_scope(f"phB_{b}"):
                        # The denominator finish of chunk c-1 rides between
                        # scores(c) and ctx(c) so PE never waits on the sum; the
                        # last chunk's finish is deferred behind phC's first conv.
                        for c in range(nch):
                            probs, psum_t = scores_exp(c)
                            if pend is not None:
                                den_prev = finish_pd(pend[0], pend[1])
                            pcx = attn_ctx(c, probs)
                            if pend is not None:
                                finish_epi(pend[0], den_prev, pend[2])
                            pend = (c, psum_t, pcx)

                # rotated schedule: conv0 of batch b+1 is emitted here, between
                # phB(b) and phC(b).  phC(b) depends on phB(b)'s DVE epilogue
                # (hA), so conv0(b+1) gives PE independent work to chew on while
                # that drains; ACT table order stays exp -> sigmoid -> sigmoid.
                if rotate and b + 1 < bpc and only_phase in (None, "A"):
                    emit_phA(b + 1)

                # ---- phase C: conv1 + proj for all chunks (sigmoid table) ----
                # proj(c-1) is emitted after conv1(c) so PE never waits on the
                # GLU DVE epilogue of chunk c before starting useful work.
                def proj_and_out(c, h2, last=False):
                    base = c * ch
                    pp = pmm.tile([IN, ch], F32, tag="mm", name="pp")
                    for kk in range(2):
                        nc.tensor.matmul(pp[:], _r(wproj[:, kk * IN:(kk + 1) * IN]),
                                         _r(h2[kk][:]), start=(kk == 0),
                                         stop=(kk == 1))
                    proj = sc2.tile([IN, ch], F32, tag="proj", name="proj")
                    nc.scalar.activation(proj[:], pp[:], AF.Identity,
                                         bias=bias[0:IN, 12:13])
                    # out stays feature-major; the host transposes it back
                    nc.sync.dma_start(out=out_d[b][:, base:base + ch],
                                      in_=proj[:])

                if only_phase in (None, "C"):
                    with nc.named_scope(f"phC_{b}"):
                        h2_prev = None
                        for c in range(nch):
                            base = c * ch
                            h2 = [sc2.tile([128, ch], F32R, tag=f"h2_{i}",
                                           name=f"h2_{i}") for i in range(2)]
                            conv_glu(w1, 6, 8, hA_buf, base,
                                     lambda i: h2[i][:],
                                     lambda i: hA_buf[:, i, 4 + base:4 + base + ch])
                            if c == 0 and pend is not None:
                                den_last = finish_pd(pend[0], pend[1])
                                finish_epi(pend[0], den_last, pend[2])
                            if h2_prev is not None:
                                proj_and_out(c - 1, h2_prev)
                            h2_prev = h2
                        proj_and_out(nch - 1, h2_prev, last=(b == bpc - 1))
        import contextlib
        loop_cm = (tc.For_i(0, loop_n, 1, hint_engines=(mybir.EngineType.PE,))
                   if loop_n > 1 else contextlib.nullcontext())
        with loop_cm:
            body_emit()

    nc.compile()
    return nc


def prep_weights(W_lin, b_lin, conv_w0, conv_b0, conv_w1, conv_b1,
                 W_attn, b_attn, W_proj, b_proj):
    def prep_conv(w):
        ws = w.astype(np.float32).copy()
        ws[D:] *= SQRT2                       # g-half
        # [512, 256, 5] -> [p, t, i, o] -> [128, 5*2*512]
        arr = ws.transpose(1, 2, 0).reshape(2, 128, 5, 2 * D).transpose(1, 2, 0, 3)
        return np.ascontiguousarray(arr.reshape(128, 5 * 2 * 2 * D))

    wlin_h = np.ascontiguousarray(W_lin.T * ISQ2).astype(np.float32)
    wproj_h = np.ascontiguousarray(
        W_proj.T.reshape(2, 128, IN).transpose(1, 0, 2).reshape(128, 2 * IN)
    ).astype(np.float32)

    bias_h = np.zeros((128, 13), np.float32)
    bias_h[:, 0] = b_lin[0:128] * ISQ2
    bias_h[:, 1] = b_lin[128:256] * ISQ2
    bias_h[:, 2] = conv_b0[0:128] * ISQ2      # a-half biases scaled
    bias_h[:, 3] = conv_b0[128:256] * ISQ2
    bias_h[:, 4] = conv_b0[256:384]           # g-half biases unscaled
    bias_h[:, 5] = conv_b0[384:512]
    bias_h[:, 6] = conv_b1[0:128] * ISQ2
    bias_h[:, 7] = conv_b1[128:256] * ISQ2
    bias_h[:, 8] = conv_b1[256:384]
    bias_h[:, 9] = conv_b1[384:512]
    bias_h[0:IN, 12] = b_proj

    return {
        "wlin": wlin_h, "w0": prep_conv(conv_w0), "w1": prep_conv(conv_w1),
        "wproj": wproj_h, "bias": bias_h,
    }


def prep_attn(enc, femb, W_attn, b_attn):
    """Host-folded attention front-end for one shard of batches.

    Token order on-device is p-outer (token = p*8 + n read "(p n)"), so the
    scores stationary (enctw) and exp bias (bshift) are pre-permuted to
    match: scores group j partition r <-> token r*8 + j.
    """
    enc = np.asarray(enc, np.float32)
    bpc, t_enc, d = enc.shape
    encsum = ((enc + np.asarray(femb, np.float32)) * ISQ2).astype(np.float32)
    # A[b][f, s] = sum_d W_attn[d, f] enc[b, s, d]  (= (enc @ W_attn)^T)
    a = np.einsum("df,bsd->bfs", np.asarray(W_attn, np.float32), enc,
                  optimize=True).astype(np.float32)
    # [b, f=i*128+p, t=r*8+j] -> [b, p, i, j, r] -> [b, p, i*1024+j*128+r]
    enctw = a.reshape(bpc, 2, 128, 128, t_enc // 128).transpose(0, 2, 1, 4, 3)
    enctw = enctw.reshape(bpc, 128, 2 * t_enc)
    bshift = (enc @ np.asarray(b_attn, np.float32) - SHIFT).astype(np.float32)
    bshift = bshift.reshape(bpc, 128, t_enc // 128)
    return {"encsum": np.ascontiguousarray(encsum),
            "enctw": np.ascontiguousarray(enctw),
            "bshift": np.ascontiguousarray(bshift)}


_NC = None


def _get_nc():
    global _NC
    if _NC is None:
        _NC = build_nc()
    return _NC


def kernel(encoder_outputs, first_embedding, mel_inputs,
           W_lin, b_lin, conv_w0, conv_b0, conv_w1, conv_b1,
           W_attn, b_attn, W_proj, b_proj):
    from concourse.bass_utils import run_bass_kernel_spmd

    nc = _get_nc()
    w = prep_weights(W_lin, b_lin, conv_w0, conv_b0, conv_w1, conv_b1,
                     W_attn, b_attn, W_proj, b_proj)
    enc = np.asarray(encoder_outputs, np.float32)
    femb = np.asarray(first_embedding, np.float32)
    mel = np.asarray(mel_inputs, np.float32)
    in_maps = []
    for c in range(NCORES):
        sl = slice(c * BPC, (c + 1) * BPC)
        in_maps.append({**prep_attn(enc[sl], femb[sl], W_attn, b_attn),
                        "mel": np.ascontiguousarray(mel[sl].transpose(0, 2, 1)),
                        **w})
    res = run_bass_kernel_spmd(nc, in_maps, list(range(NCORES)))
    out = np.concatenate([res.results[i]["out"] for i in range(NCORES)], axis=0)
    return np.ascontiguousarray(out.transpose(0, 2, 1))

